# revision 45
# baseline (speedup 1.0000x reference)
"""Per-pixel dynamic-filter 5x5 convolution (KPN-style) on 8 TRN2 NeuronCores.

Math: out[b,h,w] = sum_{di,dj,c} img[b, h+di-2, w+dj-2, c] * filts[b, h, w, (di*5+dj)*3+c]
Shapes: img [4,512,512,3] f32, filts [4,512,512,75] f32 -> out [4,512,512] f32.

Strategy (pure data parallel, no cross-core comms):
  - 8 shards = (batch b in 0..3) x (H half in 0..1); each core owns a
    [256, 512] output slab.
  - Patch-per-partition layout: partition p = hb*4 + wb owns an 8x128 pixel
    patch (hb in 0..31, wb in 0..3). Host preps img with a 2-pixel halo per
    patch: per partition [h:12][c:3][x:132] fp16, so BOTH the di and dj
    filter taps become free-dim offsets (img is DMA'd once; no shifted
    copies needed — DVE 2x mode tolerates odd element offsets). filts are
    host-transposed to [p][di][dj][h:8][c:3][w:128] fp16.
  - Per (di,dj) tap: one DVE tensor_tensor (fp16 2x mode, 3072 elems/
    partition) forms the product patch p_t[h][c][w]; the TensorEngine
    accumulates its 6 [128,512] views (3 c-planes x 2 h-halves) into two
    [128,4,128] fp32 PSUM tiles via identity matmuls (PE rides well under
    the DMA roofline). ACT evicts each half as fp16 (host upcasts), DMA out.
  - The kernel is DMA-bound: ~21 MB/core at ~350-400 GB/s. filts go as 25
    per-(di,dj) 786KB chunks to keep 3-4 DMA instructions in flight (full
    bus); the first chunk + img issue before everything else so the bus
    starts immediately; the tail-critical (4,4) chunk is delivered second
    (dedicated tile) so the DMA queue's end-of-stream straggler-engine
    trickle only delays a mid-stream group. Separate per-half PSUM tiles
    let half 0 drain to HBM while half 1 is still accumulating. Dummy
    matmuls ramp the PE clock in the first DMA's shadow.
"""

import sys

sys.path.insert(0, "/opt/trn_rl_repo")

import numpy as np

from concourse import bass, bacc, mybir
from concourse.tile import TileContext
from concourse.bass_utils import run_bass_kernel_spmd

B, H, W, C = 4, 512, 512, 3
K = 5
N_CORES = 8
HSH = H // 2  # 256 rows per shard
HB, WB = 8, 128  # patch size per partition
NHB, NWB = HSH // HB, W // WB  # 32 x 4 = 128 partitions
HH, XX = HB + 4, WB + 4  # haloed patch extents: 12 x 132
IMG_FREE = HH * C * XX  # 4752 elems per partition
N_WARMUP_MM = 10

_F16 = mybir.dt.float16
_F32 = mybir.dt.float32

_NC = None


def build_nc():
    """Build the single-core Bass program (identical on all 8 cores)."""
    nc = bacc.Bacc("TRN2")
    img_d = nc.declare_dram_parameter("img", [128, IMG_FREE], _F16, isOutput=False)
    filts_d = nc.declare_dram_parameter(
        "filts", [128, K, K, HB, C, WB], _F16, isOutput=False
    )
    ident_d = nc.declare_dram_parameter("ident", [128, 128], _F16, isOutput=False)
    out_d = nc.declare_dram_parameter("out", [128, HB, WB], _F16, isOutput=True)

    with TileContext(nc) as tc:
        with (
            tc.tile_pool(name="const", bufs=1) as constp,
            tc.tile_pool(name="imgp", bufs=1) as imgp,
            tc.tile_pool(name="filtp", bufs=4) as filtp,
            tc.tile_pool(name="ftlp", bufs=1) as ftlp,
            tc.tile_pool(name="prodp", bufs=3) as prodp,
            tc.tile_pool(name="outp", bufs=1) as outp,
            tc.tile_pool(name="psump", bufs=1, space="PSUM") as psump,
            tc.tile_pool(name="wpsump", bufs=1, space="PSUM") as wpsump,
        ):
            # first filts chunk + img first so the DMA bus starts at t=0 and
            # the first TT can begin as early as possible
            ft = {}
            for di in range(K):
                ft[di] = filtp.tile(
                    [128, K, HB, C, WB], _F16, tag="ft", name=f"ft{di}"
                )
            nc.sync.dma_start(out=ft[0][:, 0], in_=filts_d[:, 0, 0])
            img_t = imgp.tile([128, IMG_FREE], _F16)
            nc.sync.dma_start(out=img_t[:], in_=img_d[:])

            id_t = constp.tile([128, 128], _F16)
            nc.sync.dma_start(out=id_t[:], in_=ident_d[:])

            # the tail-critical (4,4) chunk is DMA'd second, into a dedicated
            # tile: the sync ring's end-of-stream straggler-engine trickle
            # then only delays a mid-group, not the psum-closing one
            ftl = ftlp.tile([128, HB, C, WB], _F16, tag="ftl", name="ftl")
            nc.sync.dma_start(out=ftl[:], in_=filts_d[:, K - 1, K - 1])

            # PE warmup in the first DMA's shadow to lift the clock p-state
            wsrc = constp.tile([128, 512], _F16, tag="wsrc")
            nc.vector.memset(wsrc[:], 0.0)
            wps = wpsump.tile([128, 512], _F32)
            for _ in range(N_WARMUP_MM):
                nc.tensor.matmul(wps[:], wsrc[:, :128], wsrc[:], start=True, stop=True)

            # per-(di,dj) filts chunks in consumption order ((4,4) already
            # issued above)
            for di in range(K):
                for dj in range(K):
                    if di == 0 and dj == 0:
                        continue
                    if di == K - 1 and dj == K - 1:
                        continue
                    nc.sync.dma_start(out=ft[di][:, dj], in_=filts_d[:, di, dj])

            # two separate PSUM tiles (one per h-half) so the final eviction
            # of half 0 can't false-serialize against half-1 matmuls
            ps_h = [
                psump.tile([128, 4, WB], _F32, tag=f"ps{hh}", name=f"ps{hh}")
                for hh in range(2)
            ]

            iv = img_t[:].rearrange("p (h c x) -> p h c x", h=HH, c=C, x=XX)
            for di in range(K):
                for dj in range(K):
                    first = di == 0 and dj == 0
                    if di == K - 1 and dj == K - 1:
                        break
                    p_t = prodp.tile(
                        [128, HB, C, WB], _F16, tag="pt", name=f"pt{di}{dj}"
                    )
                    in0 = iv[:, di : di + HB, :, dj : dj + WB]
                    nc.vector.tensor_tensor(
                        p_t[:], in0, ft[di][:, dj], mybir.AluOpType.mult
                    )
                    for c in range(C):
                        for hh in range(2):
                            nc.tensor.matmul(
                                ps_h[hh][:],
                                id_t[:],
                                p_t[:, hh * 4 : hh * 4 + 4, c, :],
                                start=(first and c == 0),
                                stop=False,
                            )

            # last group (4,4): h-split TT + hh-major matmuls so psum half 0
            # closes (and drains to HBM) while half 1 is still accumulating
            di = dj = K - 1
            for hh in range(2):
                h0 = hh * 4
                p_t = prodp.tile(
                    [128, 4, C, WB], _F16, tag=f"ptl{hh}", name=f"ptl{hh}"
                )
                in0 = iv[:, di + h0 : di + h0 + 4, :, dj : dj + WB]
                nc.vector.tensor_tensor(
                    p_t[:], in0, ftl[:, h0 : h0 + 4], mybir.AluOpType.mult
                )
                for c in range(C):
                    nc.tensor.matmul(
                        ps_h[hh][:],
                        id_t[:],
                        p_t[:, :, c, :],
                        start=False,
                        stop=(c == C - 1),
                    )
                o_t = outp.tile([128, 4, WB], _F16, tag=f"ot{hh}")
                nc.scalar.copy(out=o_t[:], in_=ps_h[hh][:])
                nc.scalar.dma_start(out=out_d[:, h0 : h0 + 4], in_=o_t[:])

    nc.compile()
    return nc


def get_nc():
    global _NC
    if _NC is None:
        _NC = build_nc()
    return _NC


def prepare_in_maps(img_stack: np.ndarray, filts: np.ndarray):
    """Shard + reformat FULL fp32 inputs into per-core fp16 input maps."""
    ident = np.eye(128, dtype=np.float16)
    in_maps = []
    img16 = img_stack.astype(np.float16)
    filts16 = filts.astype(np.float16)
    for core in range(N_CORES):
        b, hh = divmod(core, 2)
        h0 = hh * HSH
        # img patches with halo: [260, 516, 3] padded view of this shard
        pad = np.zeros((HSH + 4, W + 4, C), dtype=np.float16)
        lo, hi = max(0, h0 - 2), min(H, h0 + HSH + 2)
        pad[lo - (h0 - 2) : hi - (h0 - 2), 2 : 2 + W] = img16[b, lo:hi]
        s = pad.strides
        patches = np.lib.stride_tricks.as_strided(
            pad,
            shape=(NHB, NWB, HH, XX, C),
            strides=(HB * s[0], WB * s[1], s[0], s[1], s[2]),
        )
        img_p = np.ascontiguousarray(patches.transpose(0, 1, 2, 4, 3)).reshape(
            128, IMG_FREE
        )
        # filts -> [p, di, dj, h8, c, w128]
        f = filts16[b, h0 : h0 + HSH].reshape(NHB, HB, NWB, WB, K, K, C)
        filts_p = np.ascontiguousarray(f.transpose(0, 2, 4, 5, 1, 6, 3)).reshape(
            128, K, K, HB, C, WB
        )
        in_maps.append({"img": img_p, "filts": filts_p, "ident": ident})
    return in_maps


def assemble_out(results) -> np.ndarray:
    out = np.empty((B, H, W), dtype=np.float32)
    for core in range(N_CORES):
        b, hh = divmod(core, 2)
        o = results[core]["out"].astype(np.float32).reshape(NHB, NWB, HB, WB)
        out[b, hh * HSH : (hh + 1) * HSH, :] = o.transpose(0, 2, 1, 3).reshape(HSH, W)
    return out


def kernel(img_stack: np.ndarray, filts: np.ndarray) -> np.ndarray:
    nc = get_nc()
    in_maps = prepare_in_maps(img_stack, filts)
    res = run_bass_kernel_spmd(nc, in_maps, list(range(N_CORES)))
    return assemble_out(res.results)


# revision 47
# speedup vs baseline: 1.0206x; 1.0206x over previous
"""Per-pixel dynamic-filter 5x5 convolution (KPN-style) on 8 TRN2 NeuronCores.

Math: out[b,h,w] = sum_{di,dj,c} img[b, h+di-2, w+dj-2, c] * filts[b, h, w, (di*5+dj)*3+c]
Shapes: img [4,512,512,3] f32, filts [4,512,512,75] f32 -> out [4,512,512] f32.

Strategy (pure data parallel, no cross-core comms):
  - 8 shards = (batch b in 0..3) x (H half in 0..1); each core owns a
    [256, 512] output slab.
  - Patch-per-partition layout: partition p = hb*4 + wb owns an 8x128 pixel
    patch (hb in 0..31, wb in 0..3). Host preps img with a 2-pixel halo per
    patch: per partition [h:12][c:3][x:132] fp16, so BOTH the di and dj
    filter taps become free-dim offsets (img is DMA'd once; no shifted
    copies needed — DVE 2x mode tolerates odd element offsets). filts are
    host-transposed to [p][di][dj][h:8][c:3][w:128] fp16.
  - Per (di,dj) tap: one DVE tensor_tensor (fp16 2x mode, 3072 elems/
    partition) forms the product patch p_t[h][c][w]; the TensorEngine
    accumulates its 6 [128,512] views (3 c-planes x 2 h-halves) into two
    [128,4,128] fp32 PSUM tiles via identity matmuls (PE rides well under
    the DMA roofline). ACT evicts each half as fp16 (host upcasts), DMA out.
  - The kernel is DMA-bound: ~21 MB/core at ~350-400 GB/s. filts go as 25
    per-(di,dj) 786KB chunks to keep 3-4 DMA instructions in flight (full
    bus); the first chunk + img issue before everything else so the bus
    starts immediately; the tail-critical (4,4) chunk is delivered second
    (dedicated tile) so the DMA queue's end-of-stream straggler-engine
    trickle only delays a mid-stream group. Separate per-half PSUM tiles
    let half 0 drain to HBM while half 1 is still accumulating. Dummy
    matmuls ramp the PE clock in the first DMA's shadow.
"""

import sys

sys.path.insert(0, "/opt/trn_rl_repo")

import numpy as np

from concourse import bass, bacc, mybir
from concourse.tile import TileContext
from concourse.bass_utils import run_bass_kernel_spmd

B, H, W, C = 4, 512, 512, 3
K = 5
N_CORES = 8
HSH = H // 2  # 256 rows per shard
HB, WB = 8, 128  # patch size per partition
NHB, NWB = HSH // HB, W // WB  # 32 x 4 = 128 partitions
HH, XX = HB + 4, WB + 4  # haloed patch extents: 12 x 132
IMG_FREE = HH * C * XX  # 4752 elems per partition
N_WARMUP_MM = 10

_F16 = mybir.dt.float16
_F32 = mybir.dt.float32

_NC = None


def build_nc():
    """Build the single-core Bass program (identical on all 8 cores)."""
    nc = bacc.Bacc("TRN2")
    img_d = nc.declare_dram_parameter("img", [128, IMG_FREE], _F16, isOutput=False)
    filts_d = nc.declare_dram_parameter(
        "filts", [128, K, K, HB, C, WB], _F16, isOutput=False
    )
    ident_d = nc.declare_dram_parameter("ident", [128, 128], _F16, isOutput=False)
    out_d = nc.declare_dram_parameter("out", [128, HB, WB], _F16, isOutput=True)

    with TileContext(nc) as tc:
        with (
            tc.tile_pool(name="const", bufs=1) as constp,
            tc.tile_pool(name="imgp", bufs=1) as imgp,
            tc.tile_pool(name="filtp", bufs=4) as filtp,
            tc.tile_pool(name="ftlp", bufs=1) as ftlp,
            tc.tile_pool(name="prodp", bufs=3) as prodp,
            tc.tile_pool(name="outp", bufs=1) as outp,
            tc.tile_pool(name="psump", bufs=1, space="PSUM") as psump,
            tc.tile_pool(name="wpsump", bufs=1, space="PSUM") as wpsump,
        ):
            # first filts chunk + img first so the DMA bus starts at t=0 and
            # the first TT can begin as early as possible
            ft = {}
            for di in range(K):
                ft[di] = filtp.tile(
                    [128, K, HB, C, WB], _F16, tag="ft", name=f"ft{di}"
                )
            nc.sync.dma_start(out=ft[0][:, 0], in_=filts_d[:, 0, 0])
            img_t = imgp.tile([128, IMG_FREE], _F16)
            # img in two pieces: rows h 0..8 first so di<=1 TTs start sooner
            IMG_A = 9 * C * XX
            nc.sync.dma_start(out=img_t[:, :IMG_A], in_=img_d[:, :IMG_A])
            nc.sync.dma_start(out=img_t[:, IMG_A:], in_=img_d[:, IMG_A:])

            id_t = constp.tile([128, 128], _F16)
            nc.sync.dma_start(out=id_t[:], in_=ident_d[:])

            # the tail-critical (4,4) chunk is DMA'd second, into a dedicated
            # tile: the sync ring's end-of-stream straggler-engine trickle
            # then only delays a mid-group, not the psum-closing one
            ftl = ftlp.tile([128, HB, C, WB], _F16, tag="ftl", name="ftl")
            nc.sync.dma_start(out=ftl[:], in_=filts_d[:, K - 1, K - 1])

            # PE warmup in the first DMA's shadow to lift the clock p-state
            wsrc = constp.tile([128, 512], _F16, tag="wsrc")
            nc.vector.memset(wsrc[:], 0.0)
            wps = wpsump.tile([128, 512], _F32)
            for _ in range(N_WARMUP_MM):
                nc.tensor.matmul(wps[:], wsrc[:, :128], wsrc[:], start=True, stop=True)

            # per-(di,dj) filts chunks in consumption order ((4,4) already
            # issued above)
            for di in range(K):
                for dj in range(K):
                    if di == 0 and dj == 0:
                        continue
                    if di == K - 1 and dj == K - 1:
                        continue
                    nc.sync.dma_start(out=ft[di][:, dj], in_=filts_d[:, di, dj])

            # two separate PSUM tiles (one per h-half) so the final eviction
            # of half 0 can't false-serialize against half-1 matmuls
            ps_h = [
                psump.tile([128, 4, WB], _F32, tag=f"ps{hh}", name=f"ps{hh}")
                for hh in range(2)
            ]

            iv = img_t[:].rearrange("p (h c x) -> p h c x", h=HH, c=C, x=XX)
            # (4,4) is consumed as the SECOND group (its chunk arrives early);
            # (4,3) — the sync queue's last, crawl-delayed chunk — becomes the
            # tail group and gets the h-split treatment
            order = [(0, 0), (K - 1, K - 1)] + [
                (di, dj)
                for di in range(K)
                for dj in range(K)
                if (di, dj) not in ((0, 0), (K - 1, K - 1), (K - 1, K - 2))
            ]
            for di, dj in order:
                first = di == 0 and dj == 0
                p_t = prodp.tile(
                    [128, HB, C, WB], _F16, tag="pt", name=f"pt{di}{dj}"
                )
                in0 = iv[:, di : di + HB, :, dj : dj + WB]
                in1 = ftl[:] if (di == K - 1 and dj == K - 1) else ft[di][:, dj]
                nc.vector.tensor_tensor(p_t[:], in0, in1, mybir.AluOpType.mult)
                for c in range(C):
                    for hh in range(2):
                        nc.tensor.matmul(
                            ps_h[hh][:],
                            id_t[:],
                            p_t[:, hh * 4 : hh * 4 + 4, c, :],
                            start=(first and c == 0),
                            stop=False,
                        )

            # tail group (4,3): h-split TT + hh-major matmuls so psum half 0
            # closes (and drains to HBM) while half 1 is still accumulating
            di, dj = K - 1, K - 2
            for hh in range(2):
                h0 = hh * 4
                p_t = prodp.tile(
                    [128, 4, C, WB], _F16, tag=f"ptl{hh}", name=f"ptl{hh}"
                )
                in0 = iv[:, di + h0 : di + h0 + 4, :, dj : dj + WB]
                nc.vector.tensor_tensor(
                    p_t[:], in0, ft[di][:, dj, h0 : h0 + 4], mybir.AluOpType.mult
                )
                for c in range(C):
                    nc.tensor.matmul(
                        ps_h[hh][:],
                        id_t[:],
                        p_t[:, :, c, :],
                        start=False,
                        stop=(c == C - 1),
                    )
                o_t = outp.tile([128, 4, WB], _F16, tag=f"ot{hh}")
                nc.scalar.copy(out=o_t[:], in_=ps_h[hh][:])
                nc.scalar.dma_start(out=out_d[:, h0 : h0 + 4], in_=o_t[:])

    nc.compile()
    return nc


def get_nc():
    global _NC
    if _NC is None:
        _NC = build_nc()
    return _NC


def prepare_in_maps(img_stack: np.ndarray, filts: np.ndarray):
    """Shard + reformat FULL fp32 inputs into per-core fp16 input maps."""
    ident = np.eye(128, dtype=np.float16)
    in_maps = []
    img16 = img_stack.astype(np.float16)
    filts16 = filts.astype(np.float16)
    for core in range(N_CORES):
        b, hh = divmod(core, 2)
        h0 = hh * HSH
        # img patches with halo: [260, 516, 3] padded view of this shard
        pad = np.zeros((HSH + 4, W + 4, C), dtype=np.float16)
        lo, hi = max(0, h0 - 2), min(H, h0 + HSH + 2)
        pad[lo - (h0 - 2) : hi - (h0 - 2), 2 : 2 + W] = img16[b, lo:hi]
        s = pad.strides
        patches = np.lib.stride_tricks.as_strided(
            pad,
            shape=(NHB, NWB, HH, XX, C),
            strides=(HB * s[0], WB * s[1], s[0], s[1], s[2]),
        )
        img_p = np.ascontiguousarray(patches.transpose(0, 1, 2, 4, 3)).reshape(
            128, IMG_FREE
        )
        # filts -> [p, di, dj, h8, c, w128]
        f = filts16[b, h0 : h0 + HSH].reshape(NHB, HB, NWB, WB, K, K, C)
        filts_p = np.ascontiguousarray(f.transpose(0, 2, 4, 5, 1, 6, 3)).reshape(
            128, K, K, HB, C, WB
        )
        in_maps.append({"img": img_p, "filts": filts_p, "ident": ident})
    return in_maps


def assemble_out(results) -> np.ndarray:
    out = np.empty((B, H, W), dtype=np.float32)
    for core in range(N_CORES):
        b, hh = divmod(core, 2)
        o = results[core]["out"].astype(np.float32).reshape(NHB, NWB, HB, WB)
        out[b, hh * HSH : (hh + 1) * HSH, :] = o.transpose(0, 2, 1, 3).reshape(HSH, W)
    return out


def kernel(img_stack: np.ndarray, filts: np.ndarray) -> np.ndarray:
    nc = get_nc()
    in_maps = prepare_in_maps(img_stack, filts)
    res = run_bass_kernel_spmd(nc, in_maps, list(range(N_CORES)))
    return assemble_out(res.results)


# revision 56
# speedup vs baseline: 1.1403x; 1.1172x over previous
"""Per-pixel dynamic-filter 5x5 convolution (KPN-style) on 8 TRN2 NeuronCores.

Math: out[b,h,w] = sum_{di,dj,c} img[b, h+di-2, w+dj-2, c] * filts[b, h, w, (di*5+dj)*3+c]
Shapes: img [4,512,512,3] f32, filts [4,512,512,75] f32 -> out [4,512,512] f32.

Strategy (pure data parallel, no cross-core comms):
  - 8 shards = (batch b in 0..3) x (H half in 0..1); each core owns a
    [256, 512] output slab.
  - Patch-per-partition layout: partition p = hb*4 + wb owns an 8x128 pixel
    patch (hb in 0..31, wb in 0..3). Host preps img with a 2-pixel halo per
    patch: per partition [h:12][c:3][x:132] fp16, so BOTH the di and dj
    filter taps become free-dim offsets (img is DMA'd once; no shifted
    copies needed — DVE 2x mode tolerates odd element offsets). filts are
    host-transposed to [p][di][dj][h:8][c:3][w:128] fp16.
  - Per (di,dj) tap: one DVE tensor_tensor (fp16 2x mode, 3072 elems/
    partition) forms the product patch p_t[h][c][w]; the TensorEngine
    accumulates its 6 [128,512] views (3 c-planes x 2 h-halves) into two
    [128,4,128] fp32 PSUM tiles via identity matmuls (PE rides well under
    the DMA roofline). ACT evicts each half as fp16 (host upcasts), DMA out.
  - The kernel is DMA-bound: ~21 MB/core at ~350-400 GB/s. filts go as 25
    per-(di,dj) 786KB chunks to keep 3-4 DMA instructions in flight (full
    bus); the first chunk + img issue before everything else so the bus
    starts immediately; the tail-critical (4,4) chunk is delivered second
    (dedicated tile) so the DMA queue's end-of-stream straggler-engine
    trickle only delays a mid-stream group. Separate per-half PSUM tiles
    let half 0 drain to HBM while half 1 is still accumulating. Dummy
    matmuls ramp the PE clock in the first DMA's shadow.
"""

import sys

sys.path.insert(0, "/opt/trn_rl_repo")

import numpy as np

from concourse import bass, bacc, mybir
from concourse.tile import TileContext
from concourse.bass_utils import run_bass_kernel_spmd

B, H, W, C = 4, 512, 512, 3
K = 5
N_CORES = 8
HSH = H // 2  # 256 rows per shard
HB, WB = 8, 128  # patch size per partition
NHB, NWB = HSH // HB, W // WB  # 32 x 4 = 128 partitions
HH, XX = HB + 4, WB + 4  # haloed patch extents: 12 x 132
IMG_FREE = HH * C * XX  # 4752 elems per partition
N_WARMUP_MM = 10

_F16 = mybir.dt.float16
_F32 = mybir.dt.float32

_NC = None


def build_nc():
    """Build the single-core Bass program (identical on all 8 cores)."""
    nc = bacc.Bacc("TRN2")
    img_d = nc.declare_dram_parameter("img", [128, IMG_FREE], _F16, isOutput=False)
    filts_d = nc.declare_dram_parameter(
        "filts", [128, K, K, HB, C, WB], _F16, isOutput=False
    )
    ident_d = nc.declare_dram_parameter("ident", [128, 128], _F16, isOutput=False)
    out_d = nc.declare_dram_parameter("out", [128, HB, WB], _F16, isOutput=True)

    with TileContext(nc) as tc:
        with (
            tc.tile_pool(name="const", bufs=1) as constp,
            tc.tile_pool(name="imgp", bufs=1) as imgp,
            tc.tile_pool(name="filtp", bufs=4) as filtp,
            tc.tile_pool(name="ftlp", bufs=1) as ftlp,
            tc.tile_pool(name="prodp", bufs=3) as prodp,
            tc.tile_pool(name="outp", bufs=1) as outp,
            tc.tile_pool(name="psump", bufs=1, space="PSUM") as psump,
            tc.tile_pool(name="wpsump", bufs=1, space="PSUM") as wpsump,
        ):
            # first filts chunk + img first so the DMA bus starts at t=0 and
            # the first TT can begin as early as possible
            ft = {}
            for di in range(K):
                ft[di] = filtp.tile(
                    [128, K, HB, C, WB], _F16, tag="ft", name=f"ft{di}"
                )
            nc.sync.dma_start(out=ft[0][:, 0], in_=filts_d[:, 0, 0])
            img_t = imgp.tile([128, IMG_FREE], _F16)
            # img in two pieces: rows h 0..8 first so di<=1 TTs start sooner
            IMG_A = 9 * C * XX
            nc.sync.dma_start(out=img_t[:, :IMG_A], in_=img_d[:, :IMG_A])
            nc.sync.dma_start(out=img_t[:, IMG_A:], in_=img_d[:, IMG_A:])

            id_t = constp.tile([128, 128], _F16)
            nc.sync.dma_start(out=id_t[:], in_=ident_d[:])

            # the tail-critical (4,4) chunk is DMA'd second, into a dedicated
            # tile: the sync ring's end-of-stream straggler-engine trickle
            # then only delays a mid-group, not the psum-closing one
            ftl = ftlp.tile([128, HB, C, WB], _F16, tag="ftl", name="ftl")
            nc.sync.dma_start(out=ftl[:], in_=filts_d[:, K - 1, K - 1])

            # PE warmup in the first DMA's shadow to lift the clock p-state
            wsrc = constp.tile([128, 512], _F16, tag="wsrc")
            nc.vector.memset(wsrc[:], 0.0)
            wps = wpsump.tile([128, 512], _F32)
            for _ in range(N_WARMUP_MM):
                nc.tensor.matmul(wps[:], wsrc[:, :128], wsrc[:], start=True, stop=True)

            # per-(di,dj) filts chunks in consumption order ((4,4) already
            # issued above)
            for di in range(K):
                for dj in range(K):
                    if di == 0 and dj == 0:
                        continue
                    if di == K - 1 and dj == K - 1:
                        continue
                    nc.sync.dma_start(out=ft[di][:, dj], in_=filts_d[:, di, dj])

            # two separate PSUM tiles (one per h-half) so the final eviction
            # of half 0 can't false-serialize against half-1 matmuls
            ps_h = [
                psump.tile([128, 4, WB], _F32, tag=f"ps{hh}", name=f"ps{hh}")
                for hh in range(2)
            ]

            iv = img_t[:].rearrange("p (h c x) -> p h c x", h=HH, c=C, x=XX)
            # (4,4) is consumed as the SECOND group (its chunk arrives early);
            # (4,3) — the sync queue's last, crawl-delayed chunk — becomes the
            # tail group and gets the h-split treatment
            order = [(0, 0), (K - 1, K - 1)] + [
                (di, dj)
                for di in range(K)
                for dj in range(K)
                if (di, dj) not in ((0, 0), (K - 1, K - 1), (K - 1, K - 2))
            ]
            for di, dj in order:
                first = di == 0 and dj == 0
                p_t = prodp.tile(
                    [128, HB, C, WB], _F16, tag="pt", name=f"pt{di}{dj}"
                )
                in0 = iv[:, di : di + HB, :, dj : dj + WB]
                in1 = ftl[:] if (di == K - 1 and dj == K - 1) else ft[di][:, dj]
                nc.vector.tensor_tensor(p_t[:], in0, in1, mybir.AluOpType.mult)
                for c in range(C):
                    for hh in range(2):
                        nc.tensor.matmul(
                            ps_h[hh][:],
                            id_t[:],
                            p_t[:, hh * 4 : hh * 4 + 4, c, :],
                            start=(first and c == 0),
                            stop=False,
                        )

            # tail group (4,3): h-split TT + hh-major matmuls so psum half 0
            # closes (and drains to HBM) while half 1 is still accumulating
            di, dj = K - 1, K - 2
            for hh in range(2):
                h0 = hh * 4
                p_t = prodp.tile(
                    [128, 4, C, WB], _F16, tag=f"ptl{hh}", name=f"ptl{hh}"
                )
                in0 = iv[:, di + h0 : di + h0 + 4, :, dj : dj + WB]
                nc.vector.tensor_tensor(
                    p_t[:], in0, ft[di][:, dj, h0 : h0 + 4], mybir.AluOpType.mult
                )
                for c in range(C):
                    nc.tensor.matmul(
                        ps_h[hh][:],
                        id_t[:],
                        p_t[:, :, c, :],
                        start=False,
                        stop=(c == C - 1),
                    )
                o_t = outp.tile([128, 4, WB], _F16, tag=f"ot{hh}")
                nc.scalar.copy(out=o_t[:], in_=ps_h[hh][:])
                nc.scalar.dma_start(out=out_d[:, h0 : h0 + 4], in_=o_t[:])

    nc.compile()
    return nc


def get_nc():
    global _NC
    if _NC is None:
        _NC = build_nc()
    return _NC


def prepare_in_maps(img_stack: np.ndarray, filts: np.ndarray):
    """Shard + reformat FULL fp32 inputs into per-core fp16 input maps."""
    ident = np.eye(128, dtype=np.float16)
    in_maps = []
    img16 = img_stack.astype(np.float16)
    filts16 = filts.astype(np.float16)
    for core in range(N_CORES):
        b, hh = divmod(core, 2)
        h0 = hh * HSH
        # img patches with halo: [260, 516, 3] padded view of this shard
        pad = np.zeros((HSH + 4, W + 4, C), dtype=np.float16)
        lo, hi = max(0, h0 - 2), min(H, h0 + HSH + 2)
        pad[lo - (h0 - 2) : hi - (h0 - 2), 2 : 2 + W] = img16[b, lo:hi]
        s = pad.strides
        patches = np.lib.stride_tricks.as_strided(
            pad,
            shape=(NHB, NWB, HH, XX, C),
            strides=(HB * s[0], WB * s[1], s[0], s[1], s[2]),
        )
        img_p = np.ascontiguousarray(patches.transpose(0, 1, 2, 4, 3)).reshape(
            128, IMG_FREE
        )
        # filts -> [p, di, dj, h8, c, w128]
        f = filts16[b, h0 : h0 + HSH].reshape(NHB, HB, NWB, WB, K, K, C)
        filts_p = np.ascontiguousarray(f.transpose(0, 2, 4, 5, 1, 6, 3)).reshape(
            128, K, K, HB, C, WB
        )
        in_maps.append({"img": img_p, "filts": filts_p, "ident": ident})
    return in_maps


def assemble_out(results) -> np.ndarray:
    out = np.empty((B, H, W), dtype=np.float32)
    for core in range(N_CORES):
        b, hh = divmod(core, 2)
        o = results[core]["out"].astype(np.float32).reshape(NHB, NWB, HB, WB)
        out[b, hh * HSH : (hh + 1) * HSH, :] = o.transpose(0, 2, 1, 3).reshape(HSH, W)
    return out


def kernel(img_stack: np.ndarray, filts: np.ndarray) -> np.ndarray:
    nc = get_nc()
    in_maps = prepare_in_maps(img_stack, filts)
    res = run_bass_kernel_spmd(nc, in_maps, list(range(N_CORES)))
    return assemble_out(res.results)
